# revision 27
# baseline (speedup 1.0000x reference)
"""Trainium2 Bass kernel for BinarySplitDecoder (binary-tree leaf probabilities).

Contract: kernel(x) takes the FULL input x [65536, 1023] fp32 and returns the
FULL output [65536, 1024] fp32 (leaf probabilities of a depth-10 binary split
tree, level-major node ordering).

Sharding: pure data parallel - batch dim split evenly across 8 NeuronCores.

Per-core kernel (rows_per_core = 8192; memory-bound: ~33.5 MB of fp16 HBM
I/O against the ~420-450 GB/s 16-engine DMA wall):
  - fp16 I/O: the host converts x to fp16 and upcasts y back, halving HBM
    traffic. Tolerance is 2e-2 relative to absmax; measured end-to-end error
    of the all-fp16 pipeline is ~1.5e-3.
  - Block (bit-reversal) layout: each level writes left-children into the
    first half and right-children into the second half of the next level's
    tile, so every DVE operand has a packed (stride-1) last dim. That avoids
    the ~1.7x strided-write penalty AND qualifies every tensor_tensor for
    the DVE 2x_1p perf mode (0.52 ns/elem/partition instead of 1.04). The
    resulting column order of y is bit-reversed; the host feeds alphas
    pre-permuted per level and un-permutes y columns at the end (cheap numpy
    gathers, not device work).
  - right = cur - left replaces right = cur * (1 - a): no "1 - x" pass.
  - Two passes: levels 0-6 run ONCE for all 8192 rows (partition p owns rows
    p*64..p*64+63) in 14 large DVE ops - the per-op sequencer overhead that
    would dominate the small levels amortizes away. Levels 7-9 then run per
    row-chunk, pipelined against their alpha loads and the output stores.
    (Extending pass A to level 7 was tried and is WORSE: its 17us serial
    head delays the store pipeline and the DMA tail becomes binding.)
  - Each level group is its own DRAM param so every DMA is one fully
    contiguous block; levels 7+8 and level 9 load separately per chunk so a
    chunk can start computing once the level-7/8 alphas land.
  - Loads ride the ACT-sequencer HWDGE queue; stores alternate between the
    SP and GPSIMD queues (a single store queue caps at ~210 GB/s of
    descriptor issue and becomes the tail; two drain in parallel).
  - Small leading/trailing chunks shorten the pipeline ramp and the final
    store tail. GPSIMD never runs tensor ops (its software loops are ~10x
    slower and stall the pipeline - measured).
"""

import numpy as np

import concourse.bacc as bacc
import concourse.bass as bass
import concourse.mybir as mybir
from concourse.tile import TileContext
from concourse.bass_utils import run_bass_kernel_spmd

TREE_DEPTH = 10
N_NODES = (1 << TREE_DEPTH) - 1  # 1023
N_LEAVES = 1 << TREE_DEPTH  # 1024
N_CORES = 8
P = 128  # SBUF partitions
SPLIT_D = 7  # levels < SPLIT_D run in pass A; levels >= SPLIT_D run in pass B
NA5 = 63  # alpha columns for levels 0-5
NA6 = 64  # level 6
NB1 = 128 + 256  # levels 7 and 8
NB2 = 512  # level 9


def _bitrev(j: int, bits: int) -> int:
    r = 0
    for _ in range(bits):
        r = (r << 1) | (j & 1)
        j >>= 1
    return r


def _input_perm() -> np.ndarray:
    """perm[k] = source column of x for device column k (level-major order,
    bit-reversed node index within each level)."""
    perm = np.empty(N_NODES, dtype=np.int64)
    for d in range(TREE_DEPTH):
        base = (1 << d) - 1
        for j in range(1 << d):
            perm[base + j] = base + _bitrev(j, d)
    return perm


def _output_perm() -> np.ndarray:
    """y[:, t] = y_dev[:, outperm[t]] (bit reversal, self-inverse)."""
    return np.array([_bitrev(t, TREE_DEPTH) for t in range(N_LEAVES)], dtype=np.int64)


_IN_PERM = _input_perm()
_OUT_PERM = _output_perm()


def build_nc(rows_per_core: int, G: int = 8,
             lead: tuple = (2, 2, 4), tail: tuple = (4, 2, 2)) -> bass.Bass:
    """Build the per-core Bass program (see module docstring)."""
    U = rows_per_core // P  # row-units per partition; partition p owns
    # global rows p*U + u for u in [0, U)
    body = U - sum(lead) - sum(tail)
    assert body > 0 and body % G == 0
    chunks = list(lead) + [G] * (body // G) + list(tail)
    assert sum(chunks) == U
    f16 = mybir.dt.float16

    nc = bacc.Bacc("TRN2", target_bir_lowering=False, debug=False)
    xa5 = nc.declare_dram_parameter("xa5", [rows_per_core, NA5], f16, isOutput=False)
    xa6 = nc.declare_dram_parameter("xa6", [rows_per_core, NA6], f16, isOutput=False)
    xb1 = nc.declare_dram_parameter("xb1", [rows_per_core, NB1], f16, isOutput=False)
    xb2 = nc.declare_dram_parameter("xb2", [rows_per_core, NB2], f16, isOutput=False)
    y = nc.declare_dram_parameter("y", [rows_per_core, N_LEAVES], f16, isOutput=True)

    def full_view(t):
        return t[:, :].rearrange("(p u) n -> p (u n)", p=P, u=U)

    xb1_v = xb1.rearrange("(p u) n -> p u n", p=P, u=U)
    xb2_v = xb2.rearrange("(p u) n -> p u n", p=P, u=U)
    y_v = y.rearrange("(p u) m -> p u m", p=P, u=U)

    with TileContext(nc) as tc:
        with (
            tc.tile_pool(name="pre", bufs=1) as prep,
            tc.tile_pool(name="xin", bufs=5) as xp,
            tc.tile_pool(name="out", bufs=3) as outp,
            tc.tile_pool(name="cur", bufs=2) as curp,
        ):
            # ---- pass A: levels 0..6 for all rows, one shot ----
            xa5t = prep.tile([P, U, NA5], f16, tag="xa5")
            nc.scalar.dma_start(out=xa5t[:], in_=full_view(xa5))
            xa6t = prep.tile([P, U, NA6], f16, tag="xa6")
            nc.scalar.dma_start(out=xa6t[:], in_=full_view(xa6))

            cur = None
            for d in range(SPLIT_D):
                L = 1 << d
                nxt = prep.tile([P, U, 2 * L], f16, tag=f"pre{d % 2}")
                a = xa6t[:, :, :] if d == 6 else xa5t[:, :, L - 1 : 2 * L - 1]
                left = nxt[:, :, 0:L]
                right = nxt[:, :, L : 2 * L]
                if d == 0:
                    nc.vector.tensor_copy(out=left, in_=a)
                    nc.vector.tensor_scalar(
                        out=right,
                        in0=a,
                        scalar1=-1.0,
                        scalar2=1.0,
                        op0=mybir.AluOpType.mult,
                        op1=mybir.AluOpType.add,
                    )
                else:
                    nc.vector.tensor_mul(out=left, in0=cur, in1=a)
                    nc.vector.tensor_sub(out=right, in0=cur, in1=left)
                cur = nxt
            curA = cur  # [P, U, 128] level-6 probabilities, persists for pass B

            # ---- pass B: levels 7..9, pipelined row chunks ----
            store_q = [nc.sync, nc.gpsimd]
            u0 = 0
            for c, g in enumerate(chunks):
                xt = xp.tile([P, g, NB1], f16, tag="x1")
                nc.scalar.dma_start(out=xt[:], in_=xb1_v[:, u0 : u0 + g, :])
                xt2 = xp.tile([P, g, NB2], f16, tag="x2")
                nc.scalar.dma_start(out=xt2[:], in_=xb2_v[:, u0 : u0 + g, :])

                out_t = outp.tile([P, g, N_LEAVES], f16, tag="y")
                cur = curA[:, u0 : u0 + g, :]
                col = 0
                for d in range(SPLIT_D, TREE_DEPTH):
                    L = 1 << d
                    if d == TREE_DEPTH - 1:
                        nxt = out_t
                        a = xt2[:, :, :]
                    else:
                        nxt = curp.tile([P, g, 2 * L], f16, tag=f"cur{d % 2}")
                        a = xt[:, :, col : col + L]
                        col += L
                    left = nxt[:, :, 0:L]
                    right = nxt[:, :, L : 2 * L]
                    nc.vector.tensor_mul(out=left, in0=cur, in1=a)
                    nc.vector.tensor_sub(out=right, in0=cur, in1=left)
                    cur = nxt

                store_q[c % 2].dma_start(out=y_v[:, u0 : u0 + g, :], in_=out_t[:])
                u0 += g

    nc.compile()
    return nc


def _run(x: np.ndarray, **spmd_kwargs):
    """Shard x, run the Bass kernel on all 8 cores, return (y, BassKernelResults)."""
    x = np.asarray(x)
    B = x.shape[0]
    assert B % N_CORES == 0 and x.shape[1] == N_NODES
    rows_per_core = B // N_CORES

    # fp16 + per-level bit-reversed column order (see module docstring).
    x16 = x.astype(np.float16)[:, _IN_PERM]
    splits = np.cumsum([NA5, NA6, NB1])
    parts = np.split(x16, splits, axis=1)
    names = ["xa5", "xa6", "xb1", "xb2"]

    nc = build_nc(rows_per_core)
    core_ids = list(range(N_CORES))
    in_maps = [
        {
            nm: np.ascontiguousarray(p[i * rows_per_core : (i + 1) * rows_per_core])
            for nm, p in zip(names, parts)
        }
        for i in core_ids
    ]
    res = run_bass_kernel_spmd(nc, in_maps, core_ids, **spmd_kwargs)
    y16 = np.concatenate([r["y"] for r in res.results], axis=0)
    out = y16[:, _OUT_PERM].astype(np.float32)
    return out, res


def kernel(x: np.ndarray) -> np.ndarray:
    return _run(x)[0]


# revision 29
# speedup vs baseline: 1.0389x; 1.0389x over previous
"""Trainium2 Bass kernel for BinarySplitDecoder (binary-tree leaf probabilities).

Contract: kernel(x) takes the FULL input x [65536, 1023] fp32 and returns the
FULL output [65536, 1024] fp32 (leaf probabilities of a depth-10 binary split
tree, level-major node ordering).

Sharding: pure data parallel - batch dim split evenly across 8 NeuronCores.

Per-core kernel (rows_per_core = 8192; memory-bound: ~33.5 MB of fp16 HBM
I/O against the ~420-450 GB/s 16-engine DMA wall):
  - fp16 I/O: the host converts x to fp16 and upcasts y back, halving HBM
    traffic. Tolerance is 2e-2 relative to absmax; measured end-to-end error
    of the all-fp16 pipeline is ~1.5e-3.
  - Block (bit-reversal) layout: each level writes left-children into the
    first half and right-children into the second half of the next level's
    tile, so every DVE operand has a packed (stride-1) last dim. That avoids
    the ~1.7x strided-write penalty AND qualifies every tensor_tensor for
    the DVE 2x_1p perf mode (0.52 ns/elem/partition instead of 1.04). The
    resulting column order of y is bit-reversed; the host feeds alphas
    pre-permuted per level and un-permutes y columns at the end (cheap numpy
    gathers, not device work).
  - right = cur - left replaces right = cur * (1 - a): no "1 - x" pass.
  - Two passes: levels 0-5 run ONCE for all 8192 rows (partition p owns rows
    p*64..p*64+63) in 12 large DVE ops - the per-op sequencer overhead that
    would dominate the small levels amortizes away. Levels 6-9 (15/16 of the
    element work) then run per row-chunk, pipelined against their alpha
    loads and the output stores. (Extending pass A to level 6 or 7 was
    tried and is WORSE: the longer serial head starves the chunk pipeline
    and the DVE picks up idle gaps that outweigh the op-overhead savings.)
  - Each level group is its own DRAM param so every DMA is one fully
    contiguous block. The pass-A alphas load in two pieces (levels 0-3,
    then 4-5) so the first DVE op starts ~3us earlier; levels 6-8 and
    level 9 load separately per chunk so a chunk starts computing once its
    level 6-8 alphas land while the level-9 half is still in flight.
  - Loads ride the ACT-sequencer HWDGE queue; stores alternate between the
    SP and GPSIMD queues (a single store queue caps at ~210 GB/s of
    descriptor issue and becomes the tail; two drain in parallel and, with
    the load queue, keep all 16 DMA engines fed at 420-450 GB/s).
  - Small leading/trailing chunks shorten the pipeline ramp and the final
    store tail. GPSIMD never runs tensor ops (its software loops are ~10x
    slower and stall the pipeline - measured).
"""

import numpy as np

import concourse.bacc as bacc
import concourse.bass as bass
import concourse.mybir as mybir
from concourse.tile import TileContext
from concourse.bass_utils import run_bass_kernel_spmd

TREE_DEPTH = 10
N_NODES = (1 << TREE_DEPTH) - 1  # 1023
N_LEAVES = 1 << TREE_DEPTH  # 1024
N_CORES = 8
P = 128  # SBUF partitions
SPLIT_D = 6  # levels < SPLIT_D run in pass A; levels >= SPLIT_D run in pass B
NA3 = 15  # alpha columns for levels 0-3
NA45 = 48  # levels 4-5
NB1 = 64 + 128 + 256  # levels 6-8
NB2 = 512  # level 9


def _bitrev(j: int, bits: int) -> int:
    r = 0
    for _ in range(bits):
        r = (r << 1) | (j & 1)
        j >>= 1
    return r


def _input_perm() -> np.ndarray:
    """perm[k] = source column of x for device column k (level-major order,
    bit-reversed node index within each level)."""
    perm = np.empty(N_NODES, dtype=np.int64)
    for d in range(TREE_DEPTH):
        base = (1 << d) - 1
        for j in range(1 << d):
            perm[base + j] = base + _bitrev(j, d)
    return perm


def _output_perm() -> np.ndarray:
    """y[:, t] = y_dev[:, outperm[t]] (bit reversal, self-inverse)."""
    return np.array([_bitrev(t, TREE_DEPTH) for t in range(N_LEAVES)], dtype=np.int64)


_IN_PERM = _input_perm()
_OUT_PERM = _output_perm()


def build_nc(rows_per_core: int, G: int = 8,
             lead: tuple = (2, 2, 4), tail: tuple = (4, 2, 2)) -> bass.Bass:
    """Build the per-core Bass program (see module docstring)."""
    U = rows_per_core // P  # row-units per partition; partition p owns
    # global rows p*U + u for u in [0, U)
    body = U - sum(lead) - sum(tail)
    assert body > 0 and body % G == 0
    chunks = list(lead) + [G] * (body // G) + list(tail)
    assert sum(chunks) == U
    f16 = mybir.dt.float16

    nc = bacc.Bacc("TRN2", target_bir_lowering=False, debug=False)
    xa3 = nc.declare_dram_parameter("xa3", [rows_per_core, NA3], f16, isOutput=False)
    xa45 = nc.declare_dram_parameter("xa45", [rows_per_core, NA45], f16, isOutput=False)
    xb1 = nc.declare_dram_parameter("xb1", [rows_per_core, NB1], f16, isOutput=False)
    xb2 = nc.declare_dram_parameter("xb2", [rows_per_core, NB2], f16, isOutput=False)
    y = nc.declare_dram_parameter("y", [rows_per_core, N_LEAVES], f16, isOutput=True)

    def full_view(t):
        return t[:, :].rearrange("(p u) n -> p (u n)", p=P, u=U)

    xb1_v = xb1.rearrange("(p u) n -> p u n", p=P, u=U)
    xb2_v = xb2.rearrange("(p u) n -> p u n", p=P, u=U)
    y_v = y.rearrange("(p u) m -> p u m", p=P, u=U)

    with TileContext(nc) as tc:
        with (
            tc.tile_pool(name="pre", bufs=1) as prep,
            tc.tile_pool(name="xin", bufs=5) as xp,
            tc.tile_pool(name="out", bufs=4) as outp,
            tc.tile_pool(name="cur", bufs=2) as curp,
        ):
            # ---- pass A: levels 0..5 for all rows, one shot ----
            xa3t = prep.tile([P, U, NA3], f16, tag="xa3")
            nc.scalar.dma_start(out=xa3t[:], in_=full_view(xa3))
            xa45t = prep.tile([P, U, NA45], f16, tag="xa45")
            nc.scalar.dma_start(out=xa45t[:], in_=full_view(xa45))

            cur = None
            for d in range(SPLIT_D):
                L = 1 << d
                nxt = prep.tile([P, U, 2 * L], f16, tag=f"pre{d % 2}")
                if d < 4:
                    a = xa3t[:, :, L - 1 : 2 * L - 1]
                else:
                    a = xa45t[:, :, L - 16 : 2 * L - 16]
                left = nxt[:, :, 0:L]
                right = nxt[:, :, L : 2 * L]
                if d == 0:
                    nc.vector.tensor_copy(out=left, in_=a)
                    nc.vector.tensor_scalar(
                        out=right,
                        in0=a,
                        scalar1=-1.0,
                        scalar2=1.0,
                        op0=mybir.AluOpType.mult,
                        op1=mybir.AluOpType.add,
                    )
                else:
                    nc.vector.tensor_mul(out=left, in0=cur, in1=a)
                    nc.vector.tensor_sub(out=right, in0=cur, in1=left)
                cur = nxt
            curA = cur  # [P, U, 64] level-5 probabilities, persists for pass B

            # ---- pass B: levels 6..9, pipelined row chunks ----
            store_q = [nc.sync, nc.gpsimd]
            u0 = 0
            for c, g in enumerate(chunks):
                xt = xp.tile([P, g, NB1], f16, tag="x1")
                nc.scalar.dma_start(out=xt[:], in_=xb1_v[:, u0 : u0 + g, :])
                xt2 = xp.tile([P, g, NB2], f16, tag="x2")
                nc.scalar.dma_start(out=xt2[:], in_=xb2_v[:, u0 : u0 + g, :])

                out_t = outp.tile([P, g, N_LEAVES], f16, tag="y")
                cur = curA[:, u0 : u0 + g, :]
                col = 0
                for d in range(SPLIT_D, TREE_DEPTH):
                    L = 1 << d
                    if d == TREE_DEPTH - 1:
                        nxt = out_t
                        a = xt2[:, :, :]
                    else:
                        nxt = curp.tile([P, g, 2 * L], f16, tag=f"cur{d % 2}")
                        a = xt[:, :, col : col + L]
                        col += L
                    left = nxt[:, :, 0:L]
                    right = nxt[:, :, L : 2 * L]
                    nc.vector.tensor_mul(out=left, in0=cur, in1=a)
                    nc.vector.tensor_sub(out=right, in0=cur, in1=left)
                    cur = nxt

                store_q[c % 2].dma_start(out=y_v[:, u0 : u0 + g, :], in_=out_t[:])
                u0 += g

    nc.compile()
    return nc


def _run(x: np.ndarray, **spmd_kwargs):
    """Shard x, run the Bass kernel on all 8 cores, return (y, BassKernelResults)."""
    x = np.asarray(x)
    B = x.shape[0]
    assert B % N_CORES == 0 and x.shape[1] == N_NODES
    rows_per_core = B // N_CORES

    # fp16 + per-level bit-reversed column order (see module docstring).
    x16 = x.astype(np.float16)[:, _IN_PERM]
    splits = np.cumsum([NA3, NA45, NB1])
    parts = np.split(x16, splits, axis=1)
    names = ["xa3", "xa45", "xb1", "xb2"]

    nc = build_nc(rows_per_core)
    core_ids = list(range(N_CORES))
    in_maps = [
        {
            nm: np.ascontiguousarray(p[i * rows_per_core : (i + 1) * rows_per_core])
            for nm, p in zip(names, parts)
        }
        for i in core_ids
    ]
    res = run_bass_kernel_spmd(nc, in_maps, core_ids, **spmd_kwargs)
    y16 = np.concatenate([r["y"] for r in res.results], axis=0)
    out = y16[:, _OUT_PERM].astype(np.float32)
    return out, res


def kernel(x: np.ndarray) -> np.ndarray:
    return _run(x)[0]
